# revision 4
# baseline (speedup 1.0000x reference)
"""Multi-head attention forward kernel for Trainium2 (8 NeuronCores).

Problem: B=2, N=2048, C=1024, H=16 heads, head_dim=64.
    q = x @ Wq.T + bq  (same for k, v)
    out = softmax(q k^T / sqrt(C)) v       (per head), re-merged to [B, N, C]

Sharding: core = (batch b, head-group g): b = core // 4, g = core % 4.
Each core computes 4 heads of one batch element. No collectives needed --
outputs are disjoint; host gathers and finishes with a cheap epilogue
(normalize by the row-sums and transpose).

v2 design (from the 188us baseline's trace):
  - The EXP stream on ACT (128 ops x ~1027ns = 131us) is the hard floor;
    the kernel is organized so ACT never starves.
  - ALL matmuls run in the plain 128x128 PE mode (no tile_position
    anywhere).  The baseline mixed col-tiled PV / 128x32 ones with
    untiled QK/proj; every tiling-mode change drains the PE and killed
    MM-to-MM pipelining (every MM ran at isolated ~(398+N)/2.4 ns).
  - PV: each head's O^T accumulates at partitions 0:64 of its own PSUM
    bank (o2 tile [128, 2, 512]: head h in bank h).
  - Softmax denominators: DVE folds the two fp16 parity accumulators,
    then one M=1 ones-matmul per head into its own PSUM slot.
  - Startup: inputs are sliced into ~16-DMA waves so all 16 DMA queues
    pull concurrently (a single 512KB DMA on one queue takes ~23us);
    critical slices (wq/wk pair-0 half, x cols 0:512) come first and the
    first EXP fires at ~9us instead of ~29us.
  - Projection/V work is emitted as small half-blocks pinned to specific
    (qb, kb) windows so per-window PE work stays under the ~1us EXP.
  - PSUM budget (8 banks): st double-buffer 4 + o2 2 + shared proj/ones
    pool 2.
Outputs: out_o [2, 64, 2, N] bf16 (pair, d, head-in-pair, query) -- the
unnormalized O^T; out_s [2, 2, N] f32 (pair, head, query sums).
"""

import os
import sys

import ml_dtypes
import numpy as np

for _p in ("/opt/trn_rl_repo",):
    if _p not in sys.path:
        sys.path.insert(0, _p)

import concourse.bass as bass  # noqa: E402
import concourse.tile as tile  # noqa: E402
from concourse import bacc, mybir  # noqa: E402
from concourse.bass_utils import run_bass_kernel_spmd  # noqa: E402

N = 2048  # sequence length
C = 1024  # model dim
D = 64  # head dim
NH = 4  # heads per core
HD = NH * D  # 256 output channels per core
NCORES = 8
KB = N // 128  # 16 key chunks of 128
QB = N // 512  # 4 query blocks of 512
KC = C // 128  # 8 contraction chunks for projections
SCALE = 1.0 / 32.0  # 1 / sqrt(C)

F32 = mybir.dt.float32
BF16 = mybir.dt.bfloat16
FP16 = mybir.dt.float16

Exp = mybir.ActivationFunctionType.Exp


def build_kernel(tc, xt, wqt, wkt, wvt, bq, bk, bv, out_o, out_s):
    nc = tc.nc

    with (
        tc.tile_pool(name="res", bufs=1) as res,
        tc.tile_pool(name="stp", bufs=2, space="PSUM") as stp,
        tc.tile_pool(name="opp", bufs=1, space="PSUM") as opp,
        tc.tile_pool(name="pps", bufs=2, space="PSUM") as pps,
        tc.tile_pool(name="ptp", bufs=12) as ptp,
        tc.tile_pool(name="otp", bufs=2) as otp,
        tc.tile_pool(name="ssp", bufs=4) as ssp,
    ):
        # ---- resident SBUF tensors ----
        wq_all = res.tile([128, KC, HD], BF16, tag="wq", name="wq")
        wk_all = res.tile([128, KC, HD], BF16, tag="wk", name="wk")
        wv_all = res.tile([128, KC, HD], BF16, tag="wv", name="wv")
        xt_sb = [res.tile([128, N], BF16, tag=f"xt{k}", name=f"xt{k}") for k in range(KC)]
        wq_sb = [wq_all[:, k, :] for k in range(KC)]
        wk_sb = [wk_all[:, k, :] for k in range(KC)]
        wv_sb = [wv_all[:, k, :] for k in range(KC)]
        qt_sb = [res.tile([128, N], BF16, tag=f"qt{m}", name=f"qt{m}") for m in range(2)]
        kt_sb = [res.tile([128, N], BF16, tag=f"kt{m}", name=f"kt{m}") for m in range(2)]
        v_sb = [res.tile([128, NH, D], FP16, tag=f"v{kb}", name=f"v{kb}") for kb in range(KB)]
        bq_sb = [res.tile([128, 1], F32, tag=f"bq{m}", name=f"bq{m}") for m in range(2)]
        bk_sb = [res.tile([128, 1], F32, tag=f"bk{m}", name=f"bk{m}") for m in range(2)]
        bv_sb = res.tile([128, HD], F32, tag="bv", name="bv")
        ones_sb = res.tile([128, 1], FP16, tag="ones", name="ones")
        warm_sb = res.tile([128, 64], BF16, tag="warm", name="warm")
        warm2_sb = res.tile([1, 2], F32, tag="warm2", name="warm2")

        # ---- input DMAs in waves of ~16 so every DMA queue pulls ----
        def dma_w_half(dst_all, src, m):
            # one [C, 128] column-half of a weight, as 8 chunked DMAs
            sl = slice(m * 128, (m + 1) * 128)
            for k in range(KC):
                nc.sync.dma_start(
                    out=dst_all[:, k, sl],
                    in_=src[k * 128 : (k + 1) * 128, sl],
                )

        def dma_x_slab(j):
            # x columns [512j, 512j+512) for all 8 chunks
            sl = slice(j * 512, (j + 1) * 512)
            for k in range(KC):
                nc.sync.dma_start(out=xt_sb[k][:, sl], in_=xt[k * 128 : (k + 1) * 128, sl])

        # wave 1: pair-0 halves of Wq, Wk (critical for first projections)
        dma_w_half(wq_all, wqt, 0)
        dma_w_half(wk_all, wkt, 0)
        # wave 2: x cols 0:512 + pair-0..1 half of Wv + biases
        dma_x_slab(0)
        dma_w_half(wv_all, wvt, 0)
        for m in range(2):
            sl = slice(m * 128, (m + 1) * 128)
            nc.sync.dma_start(out=bq_sb[m][:], in_=bq[sl])
            nc.sync.dma_start(out=bk_sb[m][:], in_=bk[sl])
        bv_bcast = bass.AP(tensor=bv.tensor, offset=bv.offset, ap=[[0, 128]] + list(bv.ap))
        nc.sync.dma_start(out=bv_sb[:], in_=bv_bcast)
        # wave 3: x cols 512:1024
        dma_x_slab(1)
        # wave 4: x cols 1024:2048
        dma_x_slab(2)
        dma_x_slab(3)
        # wave 5: pair-1 weight halves
        dma_w_half(wq_all, wqt, 1)
        dma_w_half(wk_all, wkt, 1)
        dma_w_half(wv_all, wvt, 1)

        nc.vector.memset(ones_sb[:], 1.0)
        nc.vector.memset(warm_sb[:], 0.5)
        # warm up the ACT exp table while DMAs land
        nc.vector.memset(warm2_sb[:], 0.0)
        nc.scalar.activation(out=warm2_sb[:, 0:1], in_=warm2_sb[:, 1:2], func=Exp)
        # warm up the PE (HAM un-throttles after ~3.4us of activity) on
        # junk data that depends only on the memsets above
        wps = pps.tile([64, 64], F32, tag="ps", name="wps")
        for i in range(40):
            nc.tensor.matmul(
                out=wps[:],
                lhsT=warm_sb[:, 0:64],
                rhs=warm_sb[:],
                start=(i == 0),
                stop=(i == 39),
            )

        # ---- building blocks ----
        def proj_qk_half(state, which, m, nb, half):
            """Half (4 chunks) of a q/k projection block [128, 512]."""
            w_sb = wq_sb if which == "q" else wk_sb
            if half == 0:
                state["ps"] = pps.tile([128, 512], F32, tag="ps", name="qkps")
            ps = state["ps"]
            nsl = slice(nb * 512, (nb + 1) * 512)
            for k in range(4 * half, 4 * half + 4):
                nc.tensor.matmul(
                    out=ps[:],
                    lhsT=w_sb[k][:, m * 128 : (m + 1) * 128],
                    rhs=xt_sb[k][:, nsl],
                    start=(k == 0),
                    stop=(k == KC - 1),
                )
            if half == 1:
                b_sb = (bq_sb if which == "q" else bk_sb)[m]
                t_sb = (qt_sb if which == "q" else kt_sb)[m]
                nc.vector.tensor_scalar_add(out=t_sb[:, nsl], in0=ps[:], scalar1=b_sb[:])

        def proj_qk_block(which, m, nb):
            st = {}
            proj_qk_half(st, which, m, nb, 0)
            proj_qk_half(st, which, m, nb, 1)

        def proj_v_block(kb):
            vps = pps.tile([128, HD], F32, tag="ps", name="vps")
            for k in range(KC):
                nc.tensor.matmul(
                    out=vps[:],
                    lhsT=xt_sb[k][:, kb * 128 : (kb + 1) * 128],
                    rhs=wv_sb[k][:],
                    start=(k == 0),
                    stop=(k == KC - 1),
                )
            nc.vector.tensor_add(
                out=v_sb[kb][:],
                in0=vps[:].rearrange("p (h d) -> p h d", h=NH),
                in1=bv_sb[:].rearrange("p (h d) -> p h d", h=NH),
            )

        # ---- filler schedule: window w in 0..127 -> list of closures ----
        filler = {}

        def sched(w, fn):
            filler.setdefault(w, []).append(fn)

        def sched_block(w0, w1, which, m, nb):
            st = {}
            sched(w0, lambda st=st: proj_qk_half(st, which, m, nb, 0))
            sched(w1, lambda st=st: proj_qk_half(st, which, m, nb, 1))

        # pair-0 qb0: V projections just in time (v(kb) at window kb-2),
        # kt0 blocks 1-3 before their consuming kb, qt0-b1 before w16.
        for kb in range(2, KB):
            sched(kb - 2, lambda kb=kb: proj_v_block(kb))
        sched_block(1, 2, "k", 0, 1)
        sched_block(5, 6, "k", 0, 2)
        sched_block(9, 10, "k", 0, 3)
        sched_block(12, 14, "q", 0, 1)
        # pair-0 qb1: remaining qt0 blocks
        sched_block(17, 19, "q", 0, 2)
        sched_block(22, 24, "q", 0, 3)
        # pair-0 qb2: kt1 blocks
        sched_block(33, 35, "k", 1, 0)
        sched_block(37, 39, "k", 1, 1)
        sched_block(41, 43, "k", 1, 2)
        sched_block(45, 47, "k", 1, 3)
        # pair-0 qb3: qt1 blocks
        sched_block(49, 51, "q", 1, 0)
        sched_block(53, 55, "q", 1, 1)
        sched_block(57, 59, "q", 1, 2)
        sched_block(61, 63, "q", 1, 3)

        # ---- prologue projections: enough for the first EXP + v0, v1 ----
        proj_qk_block("k", 0, 0)
        proj_qk_block("q", 0, 0)
        proj_v_block(0)
        proj_v_block(1)

        # ---- main attention loop ----
        for p in range(2):
            for qb in range(QB):
                w_base = p * 64 + qb * 16
                qsl = slice(qb * 512, (qb + 1) * 512)
                # head h accumulates O^T at partitions 0:64 of bank h
                o2 = opp.tile([128, 2, 512], F32, tag="o", name="o2")
                ssum = [
                    ssp.tile([128, 2, 512], FP16, tag=f"ssum{j}", name=f"ssum{j}")
                    for j in range(2)
                ]

                def emit_pv(args, o2=o2, ssum=ssum):
                    kb, pt = args
                    for h in range(2):
                        nc.tensor.matmul(
                            out=o2[0:D, h, :],
                            lhsT=v_sb[kb][:, 2 * p + h, :],
                            rhs=pt[:, h, :],
                            start=(kb == 0),
                            stop=(kb == KB - 1),
                        )
                    sj = ssum[kb % 2]
                    if kb < 2:
                        nc.vector.tensor_copy(out=sj[:], in_=pt[:])
                    else:
                        nc.vector.tensor_add(out=sj[:], in0=sj[:], in1=pt[:])

                # PV + ssum run one kb behind the exp stream so the PE
                # never gates the next QK on o2/bank waits.
                prev = None
                for kb in range(KB):
                    ksl = slice(kb * 128, (kb + 1) * 128)
                    st = stp.tile([128, 2, 512], F32, tag="st", name="st")
                    for h in range(2):
                        hsl = slice(h * D, (h + 1) * D)
                        nc.tensor.matmul(
                            out=st[:, h, :],
                            lhsT=kt_sb[p][hsl, ksl],
                            rhs=qt_sb[p][hsl, qsl],
                            start=True,
                            stop=True,
                        )
                    pt = ptp.tile([128, 2, 512], FP16, tag="pt", name="pt")
                    nc.scalar.activation(out=pt[:], in_=st[:], func=Exp, scale=SCALE)
                    for fn in filler.get(w_base + kb, ()):
                        fn()
                    if prev is not None:
                        emit_pv(prev)
                    prev = (kb, pt)
                emit_pv(prev)

                # fold parity accumulators, then one M=1 ones-matmul per
                # head into its own PSUM slot (plain 128x128 mode)
                nc.vector.tensor_add(out=ssum[0][:], in0=ssum[0][:], in1=ssum[1][:])
                ss = otp.tile([1, 2, 512], F32, tag="ss", name="ss")
                for h in range(2):
                    s_ps = pps.tile([1, 512], F32, tag="ps", name="sps")
                    nc.tensor.matmul(
                        out=s_ps[:],
                        lhsT=ones_sb[:],
                        rhs=ssum[0][:, h, :],
                        start=True,
                        stop=True,
                    )
                    nc.vector.tensor_copy(out=ss[:, h, :], in_=s_ps[:])
                nc.sync.dma_start(out=out_s[p, :, qsl], in_=ss[:])
                ot = otp.tile([D, 2, 512], BF16, tag="ot", name="ot")
                nc.vector.tensor_copy(out=ot[:], in_=o2[0:D, :, :])
                nc.sync.dma_start(out=out_o[p, :, :, qsl], in_=ot[:])


def build_nc():
    nc = bacc.Bacc(
        "TRN2",
        target_bir_lowering=False,
        debug=False,
        num_devices=NCORES,
        enable_partition_id=False,
    )
    xt = nc.dram_tensor("xt", [C, N], BF16, kind="ExternalInput").ap()
    wqt = nc.dram_tensor("wqt", [C, HD], BF16, kind="ExternalInput").ap()
    wkt = nc.dram_tensor("wkt", [C, HD], BF16, kind="ExternalInput").ap()
    wvt = nc.dram_tensor("wvt", [C, HD], BF16, kind="ExternalInput").ap()
    bq = nc.dram_tensor("bq", [HD], F32, kind="ExternalInput").ap()
    bk = nc.dram_tensor("bk", [HD], F32, kind="ExternalInput").ap()
    bv = nc.dram_tensor("bv", [HD], F32, kind="ExternalInput").ap()
    out_o = nc.dram_tensor("out_o", [2, D, 2, N], BF16, kind="ExternalOutput").ap()
    out_s = nc.dram_tensor("out_s", [2, 2, N], F32, kind="ExternalOutput").ap()

    with tile.TileContext(nc) as tc:
        build_kernel(tc, xt, wqt, wkt, wvt, bq, bk, bv, out_o, out_s)
    nc.compile()
    return nc


def shard_inputs(inputs):
    x = np.asarray(inputs["x"], np.float32)
    in_maps = []
    for core in range(NCORES):
        b, g = core // 4, core % 4
        sl = slice(g * HD, (g + 1) * HD)
        in_maps.append(
            {
                "xt": np.ascontiguousarray(x[b].T).astype(ml_dtypes.bfloat16),
                "wqt": np.ascontiguousarray(np.asarray(inputs["Wq"], np.float32)[sl, :].T).astype(ml_dtypes.bfloat16),
                "wkt": np.ascontiguousarray(np.asarray(inputs["Wk"], np.float32)[sl, :].T).astype(ml_dtypes.bfloat16),
                "wvt": np.ascontiguousarray(np.asarray(inputs["Wv"], np.float32)[sl, :].T).astype(ml_dtypes.bfloat16),
                "bq": np.ascontiguousarray(np.asarray(inputs["bq"], np.float32)[sl]),
                "bk": np.ascontiguousarray(np.asarray(inputs["bk"], np.float32)[sl]),
                "bv": np.ascontiguousarray(np.asarray(inputs["bv"], np.float32)[sl]),
            }
        )
    return in_maps


def assemble(results, B=2):
    out = np.zeros((B, N, C), np.float32)
    for core in range(NCORES):
        b, g = core // 4, core % 4
        oo = np.asarray(results[core]["out_o"], np.float32)  # [2, D, 2, N]
        os_ = np.asarray(results[core]["out_s"], np.float32)  # [2, 2, N]
        # [pair, d, head, n] -> [pair, head, d, n], normalize
        o = oo.transpose(0, 2, 1, 3)
        on = o / os_[:, :, None, :]
        # [pair, head, d, n] -> [n, pair*2*D + head*D + d]
        out[b, :, g * HD : (g + 1) * HD] = on.transpose(3, 0, 1, 2).reshape(N, HD)
    return out


_NC_CACHE = None


def _get_nc():
    global _NC_CACHE
    if _NC_CACHE is None:
        _NC_CACHE = build_nc()
    return _NC_CACHE


def kernel(**inputs):
    nc = _get_nc()
    in_maps = shard_inputs(inputs)
    res = run_bass_kernel_spmd(
        nc,
        in_maps,
        core_ids=list(range(NCORES)),
        trace=bool(int(os.environ.get("KERNEL_TRACE", "0"))),
    )
    return assemble(res.results, B=int(np.asarray(inputs["x"]).shape[0]))


# revision 5
# speedup vs baseline: 1.3282x; 1.3282x over previous
"""Multi-head attention forward kernel for Trainium2 (8 NeuronCores).

Problem: B=2, N=2048, C=1024, H=16 heads, head_dim=64.
    q = x @ Wq.T + bq  (same for k, v)
    out = softmax(q k^T / sqrt(C)) v       (per head), re-merged to [B, N, C]

Sharding: core = (batch b, head-group g): b = core // 4, g = core % 4.
Each core computes 4 heads of one batch element. No collectives needed --
outputs are disjoint; host gathers and finishes with a cheap epilogue
(normalize by the row-sums and transpose).

Per-core design (measured ~190us: ACT-exp-bound at ~144us busy, PE ~155us,
DVE ~116us):
  - x/W matmul operands bf16 (host-converted); P/V fp16; PSUM fp32.
  - QT/KT [256, N] head-major (row-packed K=64 QK matmuls for both heads of
    a pair run concurrently on the PE); V [N, 4, 64] natural.
  - S^T chunk [128 keys, 2 heads, 512 q] fp32 PSUM (one bank per head);
    one ACT exp op covers both heads' chunks -> 128 exp ops total.
  - PV col-packed: both heads' O^T accumulate into one [128, 512] fp32 PSUM
    tile (head h at partitions h*64..h*64+63, tile_position col packing) --
    two concurrent M=64 matmuls per key chunk.
  - Softmax denominators: DVE accumulates P^T chunks into two fp16 parity
    accumulators (fast 2-byte DVE mode); ones-vector matmuls reduce over
    the 128 key partitions into PSUM partitions {0, 32} (tile_position).
  - Projection blocks are emitted inside the attention loops at the latest
    dependency-legal spot, so they fill PE idle under the ACT-bound exp
    stream instead of delaying it (emission order = scheduler priority).
  - Normalization + final transpose happen on the host (cheap epilogue).
Outputs: out_o [2, 128, N] (pair, head-major O^T rows, queries),
         out_s [2, 2, N]   (pair, head, query sums).
"""

import os
import sys

import ml_dtypes
import numpy as np

for _p in ("/opt/trn_rl_repo",):
    if _p not in sys.path:
        sys.path.insert(0, _p)

import concourse.bass as bass  # noqa: E402
import concourse.tile as tile  # noqa: E402
from concourse import bacc, mybir  # noqa: E402
from concourse.bass_utils import run_bass_kernel_spmd  # noqa: E402

N = 2048  # sequence length
C = 1024  # model dim
D = 64  # head dim
NH = 4  # heads per core
HD = NH * D  # 256 output channels per core
NCORES = 8
KB = N // 128  # 16 key chunks of 128
QB = N // 512  # 4 query blocks of 512
KC = C // 128  # 8 contraction chunks for projections
SCALE = 1.0 / 32.0  # 1 / sqrt(C)

F32 = mybir.dt.float32
BF16 = mybir.dt.bfloat16
FP16 = mybir.dt.float16


def build_kernel(tc, xt, wqt, wkt, wvt, bq, bk, bv, out_o, out_s):
    nc = tc.nc
    Exp = mybir.ActivationFunctionType.Exp

    with (
        tc.tile_pool(name="res", bufs=1) as res,
        tc.tile_pool(name="ppsum", bufs=2, space="PSUM") as ppsum,
        tc.tile_pool(name="stp", bufs=2, space="PSUM") as stp,
        tc.tile_pool(name="opp", bufs=1, space="PSUM") as opp,
        tc.tile_pool(name="sup", bufs=1, space="PSUM") as sup,
        tc.tile_pool(name="ptp", bufs=16) as ptp,
        tc.tile_pool(name="otp", bufs=2) as otp,
        tc.tile_pool(name="ssp", bufs=2) as ssp,
    ):
        # ---- resident SBUF tensors ----
        wq_all = res.tile([128, KC, HD], BF16, tag="wq", name="wq")
        wk_all = res.tile([128, KC, HD], BF16, tag="wk", name="wk")
        wv_all = res.tile([128, KC, HD], BF16, tag="wv", name="wv")
        xt_sb = [res.tile([128, N], BF16, tag=f"xt{k}", name=f"xt{k}") for k in range(KC)]
        wq_sb = [wq_all[:, k, :] for k in range(KC)]
        wk_sb = [wk_all[:, k, :] for k in range(KC)]
        wv_sb = [wv_all[:, k, :] for k in range(KC)]
        qt_sb = [res.tile([128, N], BF16, tag=f"qt{m}", name=f"qt{m}") for m in range(2)]
        kt_sb = [res.tile([128, N], BF16, tag=f"kt{m}", name=f"kt{m}") for m in range(2)]
        v_sb = [res.tile([128, NH, D], FP16, tag=f"v{kb}", name=f"v{kb}") for kb in range(KB)]
        bq_sb = [res.tile([128, 1], F32, tag=f"bq{m}", name=f"bq{m}") for m in range(2)]
        bk_sb = [res.tile([128, 1], F32, tag=f"bk{m}", name=f"bk{m}") for m in range(2)]
        bv_sb = res.tile([128, HD], F32, tag="bv", name="bv")
        ones_sb = res.tile([128, 1], FP16, tag="ones", name="ones")
        warm_sb = res.tile([1, 2], F32, tag="warm", name="warm")

        # ---- input DMAs: weights for the first projections, then x chunks ----
        nc.sync.dma_start(out=wq_all[:], in_=wqt.rearrange("(k p) n -> p k n", p=128))
        nc.sync.dma_start(out=wk_all[:], in_=wkt.rearrange("(k p) n -> p k n", p=128))
        for k in range(KC):
            nc.sync.dma_start(out=xt_sb[k][:], in_=xt[k * 128 : (k + 1) * 128, :])
        nc.sync.dma_start(out=wv_all[:], in_=wvt.rearrange("(k p) n -> p k n", p=128))
        for m in range(2):
            sl = slice(m * 128, (m + 1) * 128)
            nc.sync.dma_start(out=bq_sb[m][:], in_=bq[sl])
            nc.sync.dma_start(out=bk_sb[m][:], in_=bk[sl])
        bv_bcast = bass.AP(tensor=bv.tensor, offset=bv.offset, ap=[[0, 128]] + list(bv.ap))
        nc.sync.dma_start(out=bv_sb[:], in_=bv_bcast)
        nc.vector.memset(ones_sb[:], 1.0)
        # warm up the ACT exp table while DMAs land
        nc.vector.memset(warm_sb[:], 0.0)
        nc.scalar.activation(out=warm_sb[:, 0:1], in_=warm_sb[:, 1:2], func=Exp)

        def proj_qk_block(which, m, nb):
            w_sb = wq_sb if which == "q" else wk_sb
            b_sb = (bq_sb if which == "q" else bk_sb)[m]
            t_sb = (qt_sb if which == "q" else kt_sb)[m]
            nsl = slice(nb * 512, (nb + 1) * 512)
            ps = ppsum.tile([128, 512], F32, tag="qkps", name="qkps")
            for k in range(KC):
                nc.tensor.matmul(
                    out=ps[:],
                    lhsT=w_sb[k][:, m * 128 : (m + 1) * 128],
                    rhs=xt_sb[k][:, nsl],
                    start=(k == 0),
                    stop=(k == KC - 1),
                )
            nc.vector.tensor_scalar_add(out=t_sb[:, nsl], in0=ps[:], scalar1=b_sb[:])

        def proj_v_block(kb):
            vps = ppsum.tile([128, HD], F32, tag="qkps", name="vps")
            for k in range(KC):
                nc.tensor.matmul(
                    out=vps[:],
                    lhsT=xt_sb[k][:, kb * 128 : (kb + 1) * 128],
                    rhs=wv_sb[k][:],
                    start=(k == 0),
                    stop=(k == KC - 1),
                )
            nc.vector.tensor_add(
                out=v_sb[kb][:],
                in0=vps[:].rearrange("p (h d) -> p h d", h=NH),
                in1=bv_sb[:].rearrange("p (h d) -> p h d", h=NH),
            )

        def attn(p, pre_pv_hook=None, post_exp_hook=None):
            for qb in range(QB):
                qsl = slice(qb * 512, (qb + 1) * 512)
                # both heads' O^T col-packed: head h at partitions h*64..
                o_ps = opp.tile([128, 512], F32, tag="o", name="o")
                # running sums of P^T chunks (softmax denominators): two
                # fp16 parity accumulators keep the DVE in its fast 2-byte
                # mode and halve the accumulation depth.
                ssum = [
                    ssp.tile([128, 2, 512], FP16, tag=f"ssum{j}", name=f"ssum{j}")
                    for j in range(2)
                ]

                def emit_pv(args):
                    kb, pt = args
                    for h in range(2):
                        nc.tensor.matmul(
                            out=o_ps[h * D : (h + 1) * D, :],
                            lhsT=v_sb[kb][:, 2 * p + h, :],
                            rhs=pt[:, h, :],
                            start=(kb == 0),
                            stop=(kb == KB - 1),
                            tile_position=(0, h * D),
                            skip_group_check=True,
                        )
                    sj = ssum[kb % 2]
                    if kb < 2:
                        nc.vector.tensor_copy(out=sj[:], in_=pt[:])
                    else:
                        nc.vector.tensor_add(out=sj[:], in0=sj[:], in1=pt[:])

                # PV + ssum are emitted one kb behind their exp so the
                # in-order PE never sits on the o-psum wait before issuing
                # the next QK pair (which would stall the ACT exp pipeline).
                prev = None
                for kb in range(KB):
                    if pre_pv_hook is not None:
                        pre_pv_hook(qb, kb)
                    ksl = slice(kb * 128, (kb + 1) * 128)
                    # st layout [128 keys, head, 512 q] fp32: head h
                    # occupies its own PSUM bank -> the two concurrently-
                    # drained row-packed matmuls hit different banks.
                    st = stp.tile([128, 2, 512], F32, tag="st", name="st")
                    for h in range(2):
                        hsl = slice(h * D, (h + 1) * D)
                        nc.tensor.matmul(
                            out=st[:, h, :],
                            lhsT=kt_sb[p][hsl, ksl],
                            rhs=qt_sb[p][hsl, qsl],
                            start=True,
                            stop=True,
                        )
                    pt = ptp.tile([128, 2, 512], FP16, tag="pt", name="pt")
                    nc.scalar.activation(out=pt[:], in_=st[:], func=Exp, scale=SCALE)
                    if post_exp_hook is not None:
                        post_exp_hook(qb, kb)
                    if prev is not None:
                        emit_pv(prev)
                    prev = (kb, pt)
                emit_pv(prev)

                # partition-reduce the running sums with ones-vector
                # matmuls (both parity accumulators accumulate into the same
                # PSUM row); head h lands at PSUM partition 32*h.
                s_ps = sup.tile([33, 512], F32, tag="sps", name="sps")
                for h in range(2):
                    for j in range(2):
                        nc.tensor.matmul(
                            out=s_ps[32 * h : 32 * h + 1, :],
                            lhsT=ones_sb[:],
                            rhs=ssum[j][:, h, :],
                            start=(j == 0),
                            stop=(j == 1),
                            tile_position=(0, 32 * h),
                            skip_group_check=True,
                        )
                ss = otp.tile([33, 512], F32, tag="ss", name="ss")
                for h in range(2):
                    nc.vector.tensor_copy(
                        out=ss[32 * h : 32 * h + 1, :],
                        in_=s_ps[32 * h : 32 * h + 1, :],
                    )
                ss_view = bass.AP(
                    tensor=ss.tensor, offset=ss.offset,
                    ap=[[32 * ss.ap[0][0], 2]] + list(ss.ap[1:]),
                )
                nc.sync.dma_start(out=out_s[p, :, qsl], in_=ss_view)
                ot = otp.tile([128, 512], F32, tag="ot", name="ot")
                nc.vector.tensor_copy(out=ot[:], in_=o_ps[:])
                nc.sync.dma_start(out=out_o[p, :, qsl], in_=ot[:])

        # ---- emission order doubles as scheduler priority, and dependency
        # tracking follows emission order -- producers must precede their
        # consumers.  Q/K pair 0 first (gates the first exp), V projection
        # interleaved per-kb into attention qb0 (each v tile lands just
        # before the PV that consumes it; the pt pool decouples the ACT exp
        # stream from the lagging PV chain), Q/K pair 1 as PE filler inside
        # pair-0's ACT-bound window.
        def proj_qk_first():
            qps = ppsum.tile([128, 512], F32, tag="qkps", name="qkps")
            kps = ppsum.tile([128, 512], F32, tag="qkps", name="qkps")
            for k in range(KC):
                for w_sb, ps in ((wq_sb, qps), (wk_sb, kps)):
                    nc.tensor.matmul(
                        out=ps[:],
                        lhsT=w_sb[k][:, 0:128],
                        rhs=xt_sb[k][:, 0:512],
                        start=(k == 0),
                        stop=(k == KC - 1),
                    )
            nc.vector.tensor_scalar_add(out=qt_sb[0][:, 0:512], in0=qps[:], scalar1=bq_sb[0][:])
            nc.vector.tensor_scalar_add(out=kt_sb[0][:, 0:512], in0=kps[:], scalar1=bk_sb[0][:])

        # Filler projection blocks are interleaved into both attention
        # pairs' ACT-bound windows, each at the latest iteration that still
        # precedes (in emission = dependency order) its first consumer, so
        # exps are never gated behind unrelated projection matmuls and the
        # filler spreads over the whole kernel's PE idle time.
        def pair0_hook(qb, kb):
            if qb == 0:
                proj_v_block(kb)

        def pair0_post(qb, kb):
            if qb == 0:
                if kb == 1:
                    proj_qk_block("k", 0, 1)
                elif kb == 5:
                    proj_qk_block("k", 0, 2)
                elif kb == 9:
                    proj_qk_block("k", 0, 3)
                elif kb == 13:
                    proj_qk_block("q", 0, 1)
            elif qb == 1:
                if kb == 1:
                    proj_qk_block("q", 0, 2)
                elif kb == 9:
                    proj_qk_block("q", 0, 3)
            elif qb == 3:
                if kb == 2:
                    proj_qk_block("k", 1, 0)
                elif kb == 6:
                    proj_qk_block("q", 1, 0)

        def pair1_post(qb, kb):
            if qb == 0:
                if kb == 1:
                    proj_qk_block("k", 1, 1)
                elif kb == 5:
                    proj_qk_block("k", 1, 2)
                elif kb == 9:
                    proj_qk_block("k", 1, 3)
                elif kb == 13:
                    proj_qk_block("q", 1, 1)
            elif qb == 1:
                if kb == 1:
                    proj_qk_block("q", 1, 2)
                elif kb == 9:
                    proj_qk_block("q", 1, 3)

        proj_qk_first()
        attn(0, pre_pv_hook=pair0_hook, post_exp_hook=pair0_post)
        attn(1, post_exp_hook=pair1_post)


def build_nc():
    nc = bacc.Bacc(
        "TRN2",
        target_bir_lowering=False,
        debug=False,
        num_devices=NCORES,
        enable_partition_id=False,
    )
    xt = nc.dram_tensor("xt", [C, N], BF16, kind="ExternalInput").ap()
    wqt = nc.dram_tensor("wqt", [C, HD], BF16, kind="ExternalInput").ap()
    wkt = nc.dram_tensor("wkt", [C, HD], BF16, kind="ExternalInput").ap()
    wvt = nc.dram_tensor("wvt", [C, HD], BF16, kind="ExternalInput").ap()
    bq = nc.dram_tensor("bq", [HD], F32, kind="ExternalInput").ap()
    bk = nc.dram_tensor("bk", [HD], F32, kind="ExternalInput").ap()
    bv = nc.dram_tensor("bv", [HD], F32, kind="ExternalInput").ap()
    out_o = nc.dram_tensor("out_o", [2, 128, N], F32, kind="ExternalOutput").ap()
    out_s = nc.dram_tensor("out_s", [2, 2, N], F32, kind="ExternalOutput").ap()

    with tile.TileContext(nc) as tc:
        build_kernel(tc, xt, wqt, wkt, wvt, bq, bk, bv, out_o, out_s)
    nc.compile()
    return nc


def shard_inputs(inputs):
    x = np.asarray(inputs["x"], np.float32)
    in_maps = []
    for core in range(NCORES):
        b, g = core // 4, core % 4
        sl = slice(g * HD, (g + 1) * HD)
        in_maps.append(
            {
                "xt": np.ascontiguousarray(x[b].T).astype(ml_dtypes.bfloat16),
                "wqt": np.ascontiguousarray(np.asarray(inputs["Wq"], np.float32)[sl, :].T).astype(ml_dtypes.bfloat16),
                "wkt": np.ascontiguousarray(np.asarray(inputs["Wk"], np.float32)[sl, :].T).astype(ml_dtypes.bfloat16),
                "wvt": np.ascontiguousarray(np.asarray(inputs["Wv"], np.float32)[sl, :].T).astype(ml_dtypes.bfloat16),
                "bq": np.ascontiguousarray(np.asarray(inputs["bq"], np.float32)[sl]),
                "bk": np.ascontiguousarray(np.asarray(inputs["bk"], np.float32)[sl]),
                "bv": np.ascontiguousarray(np.asarray(inputs["bv"], np.float32)[sl]),
            }
        )
    return in_maps


def assemble(results, B=2):
    out = np.zeros((B, N, C), np.float32)
    for core in range(NCORES):
        b, g = core // 4, core % 4
        oo = np.asarray(results[core]["out_o"], np.float32)  # [2, 128, N]
        os_ = np.asarray(results[core]["out_s"], np.float32)  # [2, 2, N]
        o = oo.reshape(2, 2, D, N)  # [pair, head, d, n]
        on = o / os_[:, :, None, :]
        # [pair, head, d, n] -> [n, pair*2*D + head*D + d]
        out[b, :, g * HD : (g + 1) * HD] = (
            on.transpose(3, 0, 1, 2).reshape(N, HD)
        )
    return out


_NC_CACHE = None


def _get_nc():
    global _NC_CACHE
    if _NC_CACHE is None:
        _NC_CACHE = build_nc()
    return _NC_CACHE


def kernel(**inputs):
    nc = _get_nc()
    in_maps = shard_inputs(inputs)
    res = run_bass_kernel_spmd(
        nc,
        in_maps,
        core_ids=list(range(NCORES)),
        trace=bool(int(os.environ.get("KERNEL_TRACE", "0"))),
    )
    return assemble(res.results, B=int(np.asarray(inputs["x"]).shape[0]))



# revision 15
# speedup vs baseline: 1.5574x; 1.1726x over previous
"""Multi-head attention forward kernel for Trainium2 (8 NeuronCores).

Problem: B=2, N=2048, C=1024, H=16 heads, head_dim=64.
    q = x @ Wq.T + bq  (same for k, v)
    out = softmax(q k^T / sqrt(C)) v       (per head), re-merged to [B, N, C]

Sharding: core = (batch b, head-group g): b = core // 4, g = core % 4.
Each core computes 4 heads of one batch element. No collectives needed --
outputs are disjoint; host gathers and finishes with a cheap epilogue
(normalize by the row-sums and transpose).

Per-core design (measured ~190us: ACT-exp-bound at ~144us busy, PE ~155us,
DVE ~116us):
  - x/W matmul operands bf16 (host-converted); P/V fp16; PSUM fp32.
  - QT/KT [256, N] head-major (row-packed K=64 QK matmuls for both heads of
    a pair run concurrently on the PE); V [N, 4, 64] natural.
  - S^T chunk [128 keys, 2 heads, 512 q] fp32 PSUM (one bank per head);
    one ACT exp op covers both heads' chunks -> 128 exp ops total.
  - PV col-packed: both heads' O^T accumulate into one [128, 512] fp32 PSUM
    tile (head h at partitions h*64..h*64+63, tile_position col packing) --
    two concurrent M=64 matmuls per key chunk.
  - Softmax denominators: DVE accumulates P^T chunks into two fp16 parity
    accumulators (fast 2-byte DVE mode); ones-vector matmuls reduce over
    the 128 key partitions into PSUM partitions {0, 32} (tile_position).
  - Projection blocks are emitted inside the attention loops at the latest
    dependency-legal spot, so they fill PE idle under the ACT-bound exp
    stream instead of delaying it (emission order = scheduler priority).
  - Normalization + final transpose happen on the host (cheap epilogue).
Outputs: out_o [2, 128, N] (pair, head-major O^T rows, queries),
         out_s [2, 2, N]   (pair, head, query sums).
"""

import os
import sys

import ml_dtypes
import numpy as np

for _p in ("/opt/trn_rl_repo",):
    if _p not in sys.path:
        sys.path.insert(0, _p)

import concourse.bass as bass  # noqa: E402
import concourse.tile as tile  # noqa: E402
from concourse import bacc, mybir  # noqa: E402
from concourse.bass_utils import run_bass_kernel_spmd  # noqa: E402

N = 2048  # sequence length
C = 1024  # model dim
D = 64  # head dim
NH = 4  # heads per core
HD = NH * D  # 256 output channels per core
NCORES = 8
KB = N // 128  # 16 key chunks of 128
QB = N // 512  # 4 query blocks of 512
KC = C // 128  # 8 contraction chunks for projections
SCALE = 1.0 / 32.0  # 1 / sqrt(C)

F32 = mybir.dt.float32
BF16 = mybir.dt.bfloat16
FP16 = mybir.dt.float16


def build_kernel(tc, xt, wqt, wkt, wvt, bq, bk, bv, out_o, out_s):
    nc = tc.nc
    Exp = mybir.ActivationFunctionType.Exp

    with (
        tc.tile_pool(name="res", bufs=1) as res,
        tc.tile_pool(name="ppsum", bufs=2, space="PSUM") as ppsum,
        tc.tile_pool(name="stp", bufs=2, space="PSUM") as stp,
        tc.tile_pool(name="opp", bufs=1, space="PSUM") as opp,
        tc.tile_pool(name="sup", bufs=1, space="PSUM") as sup,
        tc.tile_pool(name="ptp", bufs=16) as ptp,
        tc.tile_pool(name="otp", bufs=2) as otp,
        tc.tile_pool(name="ssp", bufs=2) as ssp,
    ):
        # ---- resident SBUF tensors ----
        wq_all = res.tile([128, KC, HD], BF16, tag="wq", name="wq")
        wk_all = res.tile([128, KC, HD], BF16, tag="wk", name="wk")
        wv_all = res.tile([128, KC, HD], BF16, tag="wv", name="wv")
        xt_sb = [res.tile([128, N], BF16, tag=f"xt{k}", name=f"xt{k}") for k in range(KC)]
        wq_sb = [wq_all[:, k, :] for k in range(KC)]
        wk_sb = [wk_all[:, k, :] for k in range(KC)]
        wv_sb = [wv_all[:, k, :] for k in range(KC)]
        qt_sb = [res.tile([128, N], BF16, tag=f"qt{m}", name=f"qt{m}") for m in range(2)]
        kt_sb = [res.tile([128, N], BF16, tag=f"kt{m}", name=f"kt{m}") for m in range(2)]
        v_sb = [res.tile([128, NH, D], FP16, tag=f"v{kb}", name=f"v{kb}") for kb in range(KB)]
        bq_sb = [res.tile([128, 1], F32, tag=f"bq{m}", name=f"bq{m}") for m in range(2)]
        bk_sb = [res.tile([128, 1], F32, tag=f"bk{m}", name=f"bk{m}") for m in range(2)]
        bv_sb = res.tile([128, HD], F32, tag="bv", name="bv")
        ones_sb = res.tile([128, 1], FP16, tag="ones", name="ones")
        warm_sb = res.tile([1, 2], F32, tag="warm", name="warm")
        warmmm_sb = res.tile([128, 64], BF16, tag="warmmm", name="warmmm")

        # ---- input DMAs.  Each dma_start's descriptors are spread across
        # all 16 queues, so issue order is a global priority order and the
        # phase runs at aggregate HBM bandwidth.  Critical-path first:
        # weights for the first projections, then x columns in slabs (the
        # attention loop consumes keys left to right).  Slabs keep >=1KB
        # contiguous lines per partition so descriptor overhead stays low.
        nc.sync.dma_start(out=wq_all[:], in_=wqt.rearrange("(k p) n -> p k n", p=128))
        nc.sync.dma_start(out=wk_all[:], in_=wkt.rearrange("(k p) n -> p k n", p=128))
        for k in range(KC):
            nc.sync.dma_start(out=xt_sb[k][:, 0:512], in_=xt[k * 128 : (k + 1) * 128, 0:512])
        nc.sync.dma_start(out=wv_all[:], in_=wvt.rearrange("(k p) n -> p k n", p=128))
        for m in range(2):
            sl = slice(m * 128, (m + 1) * 128)
            nc.sync.dma_start(out=bq_sb[m][:], in_=bq[sl])
            nc.sync.dma_start(out=bk_sb[m][:], in_=bk[sl])
        bv_bcast = bass.AP(tensor=bv.tensor, offset=bv.offset, ap=[[0, 128]] + list(bv.ap))
        nc.sync.dma_start(out=bv_sb[:], in_=bv_bcast)
        for j0, j1 in ((512, 1024), (1024, 1536), (1536, 2048)):
            for k in range(KC):
                nc.sync.dma_start(out=xt_sb[k][:, j0:j1], in_=xt[k * 128 : (k + 1) * 128, j0:j1])
        nc.vector.memset(ones_sb[:], 1.0)
        # warm up the ACT exp table while DMAs land
        nc.vector.memset(warm_sb[:], 0.0)
        nc.scalar.activation(out=warm_sb[:, 0:1], in_=warm_sb[:, 1:2], func=Exp)
        # warm up the PE (HAM un-throttles after ~3.4us of sustained
        # activity) on junk data so the prologue projections run at 2.4GHz
        nc.vector.memset(warmmm_sb[:], 0.5)
        wps = ppsum.tile([64, 64], F32, tag="qkps", name="wps")
        for i in range(40):
            nc.tensor.matmul(
                out=wps[:],
                lhsT=warmmm_sb[:, 0:64],
                rhs=warmmm_sb[:],
                start=(i == 0),
                stop=(i == 39),
            )

        def proj_qk_part(state, which, m, nb, k0, k1):
            """Chunks [k0, k1) of a q/k projection block; drains at k1==KC."""
            w_sb = wq_sb if which == "q" else wk_sb
            nsl = slice(nb * 512, (nb + 1) * 512)
            if k0 == 0:
                state["ps"] = ppsum.tile([128, 512], F32, tag="qkps", name="qkps")
            ps = state["ps"]
            for k in range(k0, k1):
                nc.tensor.matmul(
                    out=ps[:],
                    lhsT=w_sb[k][:, m * 128 : (m + 1) * 128],
                    rhs=xt_sb[k][:, nsl],
                    start=(k == 0),
                    stop=(k == KC - 1),
                )
            if k1 == KC:
                b_sb = (bq_sb if which == "q" else bk_sb)[m]
                t_sb = (qt_sb if which == "q" else kt_sb)[m]
                nc.vector.tensor_scalar_add(out=t_sb[:, nsl], in0=ps[:], scalar1=b_sb[:])

        def proj_v_block(kb):
            vps = ppsum.tile([128, HD], F32, tag="qkps", name="vps")
            for k in range(KC):
                nc.tensor.matmul(
                    out=vps[:],
                    lhsT=xt_sb[k][:, kb * 128 : (kb + 1) * 128],
                    rhs=wv_sb[k][:],
                    start=(k == 0),
                    stop=(k == KC - 1),
                )
            nc.vector.tensor_add(
                out=v_sb[kb][:],
                in0=vps[:].rearrange("p (h d) -> p h d", h=NH),
                in1=bv_sb[:].rearrange("p (h d) -> p h d", h=NH),
            )

        def attn(p, post_exp_hook=None):
            for qb in range(QB):
                qsl = slice(qb * 512, (qb + 1) * 512)
                # both heads' O^T col-packed: head h at partitions h*64..
                o_ps = opp.tile([128, 512], F32, tag="o", name="o")
                # running sums of P^T chunks (softmax denominators): two
                # fp16 parity accumulators keep the DVE in its fast 2-byte
                # mode and halve the accumulation depth.
                ssum = [
                    ssp.tile([128, 2, 512], FP16, tag=f"ssum{j}", name=f"ssum{j}")
                    for j in range(2)
                ]

                def emit_pv(args):
                    kb, pt = args
                    for h in range(2):
                        nc.tensor.matmul(
                            out=o_ps[h * D : (h + 1) * D, :],
                            lhsT=v_sb[kb][:, 2 * p + h, :],
                            rhs=pt[:, h, :],
                            start=(kb == 0),
                            stop=(kb == KB - 1),
                            tile_position=(0, h * D),
                            skip_group_check=True,
                        )
                    sj = ssum[kb % 2]
                    if kb < 2:
                        nc.vector.tensor_copy(out=sj[:], in_=pt[:])
                    else:
                        nc.vector.tensor_add(out=sj[:], in0=sj[:], in1=pt[:])

                # PV + ssum are emitted one kb behind their exp so the
                # in-order PE never sits on the o-psum wait before issuing
                # the next QK pair (which would stall the ACT exp pipeline).
                prev = None
                for kb in range(KB):
                    ksl = slice(kb * 128, (kb + 1) * 128)
                    # st layout [128 keys, head, 512 q] fp32: head h
                    # occupies its own PSUM bank -> the two concurrently-
                    # drained row-packed matmuls hit different banks.
                    st = stp.tile([128, 2, 512], F32, tag="st", name="st")
                    for h in range(2):
                        hsl = slice(h * D, (h + 1) * D)
                        nc.tensor.matmul(
                            out=st[:, h, :],
                            lhsT=kt_sb[p][hsl, ksl],
                            rhs=qt_sb[p][hsl, qsl],
                            start=True,
                            stop=True,
                        )
                    pt = ptp.tile([128, 2, 512], FP16, tag="pt", name="pt")
                    nc.scalar.activation(out=pt[:], in_=st[:], func=Exp, scale=SCALE)
                    if post_exp_hook is not None:
                        post_exp_hook(qb, kb)
                    if prev is not None:
                        emit_pv(prev)
                    prev = (kb, pt)
                emit_pv(prev)

                # partition-reduce the running sums with ones-vector
                # matmuls (both parity accumulators accumulate into the same
                # PSUM row); head h lands at PSUM partition 32*h.
                s_ps = sup.tile([33, 512], F32, tag="sps", name="sps")
                for h in range(2):
                    for j in range(2):
                        nc.tensor.matmul(
                            out=s_ps[32 * h : 32 * h + 1, :],
                            lhsT=ones_sb[:],
                            rhs=ssum[j][:, h, :],
                            start=(j == 0),
                            stop=(j == 1),
                            tile_position=(0, 32 * h),
                            skip_group_check=True,
                        )
                ss = otp.tile([33, 512], F32, tag="ss", name="ss")
                for h in range(2):
                    nc.vector.tensor_copy(
                        out=ss[32 * h : 32 * h + 1, :],
                        in_=s_ps[32 * h : 32 * h + 1, :],
                    )
                ss_view = bass.AP(
                    tensor=ss.tensor, offset=ss.offset,
                    ap=[[32 * ss.ap[0][0], 2]] + list(ss.ap[1:]),
                )
                nc.sync.dma_start(out=out_s[p, :, qsl], in_=ss_view)
                ot = otp.tile([128, 512], BF16, tag="ot", name="ot")
                nc.vector.tensor_copy(out=ot[:], in_=o_ps[:])
                nc.sync.dma_start(out=out_o[p, :, qsl], in_=ot[:])

        # ---- emission order doubles as scheduler priority, and dependency
        # tracking follows emission order -- producers must precede their
        # consumers.  Q/K pair 0 first (gates the first exp), V projection
        # interleaved per-kb into attention qb0 (each v tile lands just
        # before the PV that consumes it; the pt pool decouples the ACT exp
        # stream from the lagging PV chain), Q/K pair 1 as PE filler inside
        # pair-0's ACT-bound window.
        def proj_qk_first():
            qps = ppsum.tile([128, 512], F32, tag="qkps", name="qkps")
            kps = ppsum.tile([128, 512], F32, tag="qkps", name="qkps")
            for k in range(KC):
                for w_sb, ps in ((wq_sb, qps), (wk_sb, kps)):
                    nc.tensor.matmul(
                        out=ps[:],
                        lhsT=w_sb[k][:, 0:128],
                        rhs=xt_sb[k][:, 0:512],
                        start=(k == 0),
                        stop=(k == KC - 1),
                    )
            nc.vector.tensor_scalar_add(out=qt_sb[0][:, 0:512], in0=qps[:], scalar1=bq_sb[0][:])
            nc.vector.tensor_scalar_add(out=kt_sb[0][:, 0:512], in0=kps[:], scalar1=bk_sb[0][:])

        # Filler is spread at fine granularity (a 2-4 matmul projection
        # part or one 8-matmul v block per window) so per-window PE work
        # stays near the ~1.03us EXP and the ACT stream rarely starves.
        # Pair-1 windows carry no filler at all.  V(kb+1) is emitted after
        # exp(kb) so it is ready for PV(kb+1) one window later; kt0/v
        # blocks chase the x column-slab DMAs.  Only one qk block is in
        # flight at a time (it shares the 2-slot proj PSUM pool with the
        # per-window v blocks).
        filler = {}

        def sched_parts(windows, which, m, nb):
            st = {}
            bounds = [round(i * KC / len(windows)) for i in range(len(windows) + 1)]
            for w, k0, k1 in zip(windows, bounds, bounds[1:]):
                filler.setdefault(w, []).append(
                    lambda st=st, k0=k0, k1=k1: proj_qk_part(st, which, m, nb, k0, k1)
                )

        for kb in range(0, 15):
            filler.setdefault(kb, []).append(lambda kb=kb: proj_v_block(kb + 1))
        sched_parts([0, 1, 2, 3], "k", 0, 1)
        sched_parts([6, 7], "k", 0, 2)  # waits on x cols 1024:1536
        sched_parts([10, 11], "k", 0, 3)  # waits on x cols 1536:2048
        sched_parts([13, 14], "q", 0, 1)
        sched_parts([16, 17, 18, 19], "q", 0, 2)
        sched_parts([20, 21, 22, 23], "q", 0, 3)
        sched_parts([24, 25, 26, 27], "k", 1, 0)
        sched_parts([28, 29, 30, 31], "k", 1, 1)
        sched_parts([32, 33, 34, 35], "k", 1, 2)
        sched_parts([36, 37, 38, 39], "k", 1, 3)
        sched_parts([40, 41, 42, 43], "q", 1, 0)
        sched_parts([44, 45, 46, 47], "q", 1, 1)
        sched_parts([48, 49, 50, 51], "q", 1, 2)
        sched_parts([52, 53, 54, 55], "q", 1, 3)

        def pair0_post(qb, kb):
            for fn in filler.get(qb * 16 + kb, ()):
                fn()

        proj_qk_first()
        proj_v_block(0)
        attn(0, post_exp_hook=pair0_post)
        attn(1)


def build_nc():
    nc = bacc.Bacc(
        "TRN2",
        target_bir_lowering=False,
        debug=False,
        num_devices=NCORES,
        enable_partition_id=False,
    )
    xt = nc.dram_tensor("xt", [C, N], BF16, kind="ExternalInput").ap()
    wqt = nc.dram_tensor("wqt", [C, HD], BF16, kind="ExternalInput").ap()
    wkt = nc.dram_tensor("wkt", [C, HD], BF16, kind="ExternalInput").ap()
    wvt = nc.dram_tensor("wvt", [C, HD], BF16, kind="ExternalInput").ap()
    bq = nc.dram_tensor("bq", [HD], F32, kind="ExternalInput").ap()
    bk = nc.dram_tensor("bk", [HD], F32, kind="ExternalInput").ap()
    bv = nc.dram_tensor("bv", [HD], F32, kind="ExternalInput").ap()
    out_o = nc.dram_tensor("out_o", [2, 128, N], BF16, kind="ExternalOutput").ap()
    out_s = nc.dram_tensor("out_s", [2, 2, N], F32, kind="ExternalOutput").ap()

    with tile.TileContext(nc) as tc:
        build_kernel(tc, xt, wqt, wkt, wvt, bq, bk, bv, out_o, out_s)
    nc.compile()
    return nc


def shard_inputs(inputs):
    x = np.asarray(inputs["x"], np.float32)
    in_maps = []
    for core in range(NCORES):
        b, g = core // 4, core % 4
        sl = slice(g * HD, (g + 1) * HD)
        in_maps.append(
            {
                "xt": np.ascontiguousarray(x[b].T).astype(ml_dtypes.bfloat16),
                "wqt": np.ascontiguousarray(np.asarray(inputs["Wq"], np.float32)[sl, :].T).astype(ml_dtypes.bfloat16),
                "wkt": np.ascontiguousarray(np.asarray(inputs["Wk"], np.float32)[sl, :].T).astype(ml_dtypes.bfloat16),
                "wvt": np.ascontiguousarray(np.asarray(inputs["Wv"], np.float32)[sl, :].T).astype(ml_dtypes.bfloat16),
                "bq": np.ascontiguousarray(np.asarray(inputs["bq"], np.float32)[sl]),
                "bk": np.ascontiguousarray(np.asarray(inputs["bk"], np.float32)[sl]),
                "bv": np.ascontiguousarray(np.asarray(inputs["bv"], np.float32)[sl]),
            }
        )
    return in_maps


def assemble(results, B=2):
    out = np.zeros((B, N, C), np.float32)
    for core in range(NCORES):
        b, g = core // 4, core % 4
        oo = np.asarray(results[core]["out_o"], np.float32)  # [2, 128, N]
        os_ = np.asarray(results[core]["out_s"], np.float32)  # [2, 2, N]
        o = oo.reshape(2, 2, D, N)  # [pair, head, d, n]
        on = o / os_[:, :, None, :]
        # [pair, head, d, n] -> [n, pair*2*D + head*D + d]
        out[b, :, g * HD : (g + 1) * HD] = (
            on.transpose(3, 0, 1, 2).reshape(N, HD)
        )
    return out


_NC_CACHE = None


def _get_nc():
    global _NC_CACHE
    if _NC_CACHE is None:
        _NC_CACHE = build_nc()
    return _NC_CACHE


def kernel(**inputs):
    nc = _get_nc()
    in_maps = shard_inputs(inputs)
    res = run_bass_kernel_spmd(
        nc,
        in_maps,
        core_ids=list(range(NCORES)),
        trace=bool(int(os.environ.get("KERNEL_TRACE", "0"))),
    )
    return assemble(res.results, B=int(np.asarray(inputs["x"]).shape[0]))



# revision 18
# speedup vs baseline: 1.5698x; 1.0080x over previous
"""Multi-head attention forward kernel for Trainium2 (8 NeuronCores).

Problem: B=2, N=2048, C=1024, H=16 heads, head_dim=64.
    q = x @ Wq.T + bq  (same for k, v)
    out = softmax(q k^T / sqrt(C)) v       (per head), re-merged to [B, N, C]

Sharding: core = (batch b, head-group g): b = core // 4, g = core % 4.
Each core computes 4 heads of one batch element. No collectives needed --
outputs are disjoint; host gathers and finishes with a cheap epilogue
(normalize by the row-sums and transpose).

v4 design (trace-driven evolution of the 188us baseline):
  - The EXP stream on ACT (128 ops x ~1.05us) is the hard floor; the
    kernel is one flat 128-window pipeline (window = one key chunk of one
    (pair, query-block)): QK pair -> EXP -> static filler -> lagged PV.
  - PV/ssum trail the exp stream by L=6 windows and cross query-block
    boundaries (two O^T PSUM accumulators in flight), so the PE overflow
    of the v-projection-heavy first windows spreads into pair-1's slack
    instead of starving ACT.
  - Startup: weights arrive pair-split and pre-packed in SBUF layout
    (one contiguous 256KB DMA each); x columns stream in 256/512-col
    slabs.  The first q/k projections chase the per-chunk DMAs, so the
    first EXP fires at ~13us instead of ~29us.  DMA issue order is the
    priority order (descriptors of each dma_start spread over all 16
    queues; the phase runs at aggregate HBM bandwidth).
  - kt0 blocks are computed in N=256 column sub-blocks chasing the x
    slabs; remaining projection blocks are spread as small parts over
    windows with PE slack (pair-1 carries the late qt1 blocks only).
  - Softmax denominators: DVE folds the two fp16 parity accumulators,
    then one ones-matmul per head (PSUM partitions {0,32} via
    tile_position) -- half the baseline's reduction matmuls.
  - PSUM budget (8 banks): st double-buffer 4 + two O^T accumulators 2 +
    shared proj/ones pool 2.
Outputs: out_o [2, 128, N] bf16 (pair, head-major O^T rows, queries),
         out_s [2, 2, N]   f32 (pair, head, query sums).
"""

import os
import sys

import ml_dtypes
import numpy as np

for _p in ("/opt/trn_rl_repo",):
    if _p not in sys.path:
        sys.path.insert(0, _p)

import concourse.bass as bass  # noqa: E402
import concourse.tile as tile  # noqa: E402
from concourse import bacc, mybir  # noqa: E402
from concourse.bass_utils import run_bass_kernel_spmd  # noqa: E402

N = 2048  # sequence length
C = 1024  # model dim
D = 64  # head dim
NH = 4  # heads per core
HD = NH * D  # 256 output channels per core
NCORES = 8
KB = N // 128  # 16 key chunks of 128
QB = N // 512  # 4 query blocks of 512
KC = C // 128  # 8 contraction chunks for projections
SCALE = 1.0 / 32.0  # 1 / sqrt(C)
LAG = 6  # PV/ssum windows behind the exp stream

F32 = mybir.dt.float32
BF16 = mybir.dt.bfloat16
FP16 = mybir.dt.float16


def build_kernel(tc, xt, wq_d, wk_d, wv_d, bq, bk, bv, out_o, out_s):
    nc = tc.nc
    Exp = mybir.ActivationFunctionType.Exp

    with (
        tc.tile_pool(name="res", bufs=1) as res,
        tc.tile_pool(name="ppsum", bufs=2, space="PSUM") as ppsum,
        tc.tile_pool(name="stp", bufs=2, space="PSUM") as stp,
        tc.tile_pool(name="opp", bufs=2, space="PSUM") as opp,
        tc.tile_pool(name="ptp", bufs=12) as ptp,
        tc.tile_pool(name="otp", bufs=2) as otp,
        tc.tile_pool(name="ssp", bufs=2) as ssp,
    ):
        # ---- resident SBUF tensors ----
        # weights arrive pre-packed per head-pair: [128, KC, 128]
        wq_p = [res.tile([128, KC, 128], BF16, tag=f"wq{m}", name=f"wq{m}") for m in range(2)]
        wk_p = [res.tile([128, KC, 128], BF16, tag=f"wk{m}", name=f"wk{m}") for m in range(2)]
        wv_all = res.tile([128, KC, HD], BF16, tag="wv", name="wv")
        xt_sb = [res.tile([128, N], BF16, tag=f"xt{k}", name=f"xt{k}") for k in range(KC)]
        wv_sb = [wv_all[:, k, :] for k in range(KC)]
        qt_sb = [res.tile([128, N], BF16, tag=f"qt{m}", name=f"qt{m}") for m in range(2)]
        kt_sb = [res.tile([128, N], BF16, tag=f"kt{m}", name=f"kt{m}") for m in range(2)]
        v_sb = [res.tile([128, NH, D], FP16, tag=f"v{kb}", name=f"v{kb}") for kb in range(KB)]
        bq_sb = [res.tile([128, 1], F32, tag=f"bq{m}", name=f"bq{m}") for m in range(2)]
        bk_sb = [res.tile([128, 1], F32, tag=f"bk{m}", name=f"bk{m}") for m in range(2)]
        bv_sb = res.tile([128, HD], F32, tag="bv", name="bv")
        ones_sb = res.tile([128, 1], FP16, tag="ones", name="ones")
        warm_sb = res.tile([1, 2], F32, tag="warm", name="warm")
        warmmm_sb = res.tile([128, 64], BF16, tag="warmmm", name="warmmm")

        # ---- input DMAs in strict priority order ----
        nc.sync.dma_start(out=wq_p[0][:], in_=wq_d[0])
        nc.sync.dma_start(out=wk_p[0][:], in_=wk_d[0])
        for k in range(KC):
            nc.sync.dma_start(out=xt_sb[k][:, 0:512], in_=xt[k * 128 : (k + 1) * 128, 0:512])
        for m in range(2):
            sl = slice(m * 128, (m + 1) * 128)
            nc.sync.dma_start(out=bq_sb[m][:], in_=bq[sl])
            nc.sync.dma_start(out=bk_sb[m][:], in_=bk[sl])
        bv_bcast = bass.AP(tensor=bv.tensor, offset=bv.offset, ap=[[0, 128]] + list(bv.ap))
        nc.sync.dma_start(out=bv_sb[:], in_=bv_bcast)
        nc.sync.dma_start(out=wv_all[:], in_=wv_d.rearrange("(k p) n -> p k n", p=128))
        for j in range(2, 8):  # x columns 512:2048 in 256-col slabs
            for k in range(KC):
                nc.sync.dma_start(
                    out=xt_sb[k][:, j * 256 : (j + 1) * 256],
                    in_=xt[k * 128 : (k + 1) * 128, j * 256 : (j + 1) * 256],
                )
        nc.sync.dma_start(out=wq_p[1][:], in_=wq_d[1])
        nc.sync.dma_start(out=wk_p[1][:], in_=wk_d[1])

        nc.vector.memset(ones_sb[:], 1.0)
        # warm up the ACT exp table while DMAs land
        nc.vector.memset(warm_sb[:], 0.0)
        nc.scalar.activation(out=warm_sb[:, 0:1], in_=warm_sb[:, 1:2], func=Exp)
        # warm up the PE (HAM un-throttles after ~3.4us of sustained
        # activity) on junk data so the prologue projections run at 2.4GHz
        nc.vector.memset(warmmm_sb[:], 0.5)
        wps = ppsum.tile([64, 64], F32, tag="qkps", name="wps")
        for i in range(28):
            nc.tensor.matmul(
                out=wps[:],
                lhsT=warmmm_sb[:, 0:64],
                rhs=warmmm_sb[:],
                start=(i == 0),
                stop=(i == 27),
            )

        # ---- building blocks ----
        def proj_qk_part(state, which, m, nb, k0, k1):
            """Chunks [k0, k1) of a q/k projection block [128, 512]."""
            w_p = (wq_p if which == "q" else wk_p)[m]
            nsl = slice(nb * 512, (nb + 1) * 512)
            if k0 == 0:
                state["ps"] = ppsum.tile([128, 512], F32, tag="qkps", name="qkps")
            ps = state["ps"]
            for k in range(k0, k1):
                nc.tensor.matmul(
                    out=ps[:],
                    lhsT=w_p[:, k, :],
                    rhs=xt_sb[k][:, nsl],
                    start=(k == 0),
                    stop=(k == KC - 1),
                )
            if k1 == KC:
                b_sb = (bq_sb if which == "q" else bk_sb)[m]
                t_sb = (qt_sb if which == "q" else kt_sb)[m]
                nc.vector.tensor_scalar_add(out=t_sb[:, nsl], in0=ps[:], scalar1=b_sb[:])

        def proj_qk_256(which, m, nb2):
            """One N=256 column sub-block of a q/k projection (chases the
            256-col x slabs)."""
            w_p = (wq_p if which == "q" else wk_p)[m]
            nsl = slice(nb2 * 256, (nb2 + 1) * 256)
            ps = ppsum.tile([128, 256], F32, tag="qkps", name="qkps2")
            for k in range(KC):
                nc.tensor.matmul(
                    out=ps[:],
                    lhsT=w_p[:, k, :],
                    rhs=xt_sb[k][:, nsl],
                    start=(k == 0),
                    stop=(k == KC - 1),
                )
            b_sb = (bq_sb if which == "q" else bk_sb)[m]
            t_sb = (qt_sb if which == "q" else kt_sb)[m]
            nc.vector.tensor_scalar_add(out=t_sb[:, nsl], in0=ps[:], scalar1=b_sb[:])

        def proj_v_block(kb):
            vps = ppsum.tile([128, HD], F32, tag="qkps", name="vps")
            for k in range(KC):
                nc.tensor.matmul(
                    out=vps[:],
                    lhsT=xt_sb[k][:, kb * 128 : (kb + 1) * 128],
                    rhs=wv_sb[k][:],
                    start=(k == 0),
                    stop=(k == KC - 1),
                )
            nc.vector.tensor_add(
                out=v_sb[kb][:],
                in0=vps[:].rearrange("p (h d) -> p h d", h=NH),
                in1=bv_sb[:].rearrange("p (h d) -> p h d", h=NH),
            )

        # ---- static filler schedule: window -> list of closures ----
        filler = {}

        def sched(w, fn):
            filler.setdefault(w, []).append(fn)

        def sched_parts(windows, which, m, nb):
            st = {}
            bounds = [round(i * KC / len(windows)) for i in range(len(windows) + 1)]
            for w, k0, k1 in zip(windows, bounds, bounds[1:]):
                sched(w, lambda st=st, k0=k0, k1=k1: proj_qk_part(st, which, m, nb, k0, k1))

        for kb in range(1, KB):  # v(kb) three windows ahead of its PV
            sched(kb + 3, lambda kb=kb: proj_v_block(kb))
        # kt0 column sub-blocks chase the x slab DMAs
        for w, nb2 in ((1, 2), (2, 3), (5, 4), (6, 5), (9, 6), (10, 7)):
            sched(w, lambda nb2=nb2: proj_qk_256("k", 0, nb2))
        sched_parts([12, 13], "q", 0, 1)
        sched_parts([20, 21, 22, 23], "q", 0, 2)
        sched_parts([33, 34, 35, 36], "q", 0, 3)
        sched_parts([38, 39, 40, 41], "k", 1, 0)
        sched_parts([43, 44, 45, 46], "k", 1, 1)
        sched_parts([48, 49, 50, 51], "k", 1, 2)
        sched_parts([53, 54, 55, 56], "k", 1, 3)
        sched_parts([58, 59, 60, 61], "q", 1, 0)
        sched_parts([72, 73, 74, 75], "q", 1, 1)
        sched_parts([88, 89, 90, 91], "q", 1, 2)
        sched_parts([104, 105, 106, 107], "q", 1, 3)

        # ---- prologue: first projections chase the per-chunk x DMAs ----
        k0ps = ppsum.tile([128, 512], F32, tag="qkps", name="k0ps")
        q0ps = ppsum.tile([128, 512], F32, tag="qkps", name="q0ps")
        for k in range(KC):
            nc.tensor.matmul(
                out=k0ps[:], lhsT=wk_p[0][:, k, :], rhs=xt_sb[k][:, 0:512],
                start=(k == 0), stop=(k == KC - 1),
            )
            nc.tensor.matmul(
                out=q0ps[:], lhsT=wq_p[0][:, k, :], rhs=xt_sb[k][:, 0:512],
                start=(k == 0), stop=(k == KC - 1),
            )
        nc.vector.tensor_scalar_add(out=kt_sb[0][:, 0:512], in0=k0ps[:], scalar1=bk_sb[0][:])
        nc.vector.tensor_scalar_add(out=qt_sb[0][:, 0:512], in0=q0ps[:], scalar1=bq_sb[0][:])
        proj_v_block(0)

        # ---- the flat lagged window pipeline ----
        qstate = {}

        def emit_pv(p, qb, kb, pt):
            s = qstate[(p, qb)]
            o_ps = s["o"]
            for h in range(2):
                nc.tensor.matmul(
                    out=o_ps[h * D : (h + 1) * D, :],
                    lhsT=v_sb[kb][:, 2 * p + h, :],
                    rhs=pt[:, h, :],
                    start=(kb == 0),
                    stop=(kb == KB - 1),
                    tile_position=(0, h * D),
                    skip_group_check=True,
                )
            sj = s["ssum"][kb % 2]
            if kb < 2:
                nc.vector.tensor_copy(out=sj[:], in_=pt[:])
            else:
                nc.vector.tensor_add(out=sj[:], in0=sj[:], in1=pt[:])

        def epilogue(p, qb):
            s = qstate.pop((p, qb))
            qsl = slice(qb * 512, (qb + 1) * 512)
            ssum = s["ssum"]
            nc.vector.tensor_add(out=ssum[0][:], in0=ssum[0][:], in1=ssum[1][:])
            s_ps = ppsum.tile([33, 512], F32, tag="qkps", name="sps")
            for h in range(2):
                nc.tensor.matmul(
                    out=s_ps[32 * h : 32 * h + 1, :],
                    lhsT=ones_sb[:],
                    rhs=ssum[0][:, h, :],
                    start=True,
                    stop=True,
                    tile_position=(0, 32 * h),
                    skip_group_check=True,
                )
            ss = otp.tile([33, 512], F32, tag="ss", name="ss")
            for h in range(2):
                nc.vector.tensor_copy(
                    out=ss[32 * h : 32 * h + 1, :], in_=s_ps[32 * h : 32 * h + 1, :]
                )
            ss_view = bass.AP(
                tensor=ss.tensor, offset=ss.offset,
                ap=[[32 * ss.ap[0][0], 2]] + list(ss.ap[1:]),
            )
            nc.sync.dma_start(out=out_s[p, :, qsl], in_=ss_view)
            ot = otp.tile([128, 512], BF16, tag="ot", name="ot")
            nc.vector.tensor_copy(out=ot[:], in_=s["o"][:])
            nc.sync.dma_start(out=out_o[p, :, qsl], in_=ot[:])

        windows = [(p, qb, kb) for p in range(2) for qb in range(QB) for kb in range(KB)]
        pending = {}
        for w in range(len(windows) + LAG):
            if w < len(windows):
                p, qb, kb = windows[w]
                qsl = slice(qb * 512, (qb + 1) * 512)
                ksl = slice(kb * 128, (kb + 1) * 128)
                st = stp.tile([128, 2, 512], F32, tag="st", name="st")
                for h in range(2):
                    hsl = slice(h * D, (h + 1) * D)
                    nc.tensor.matmul(
                        out=st[:, h, :],
                        lhsT=kt_sb[p][hsl, ksl],
                        rhs=qt_sb[p][hsl, qsl],
                        start=True,
                        stop=True,
                    )
                pt = ptp.tile([128, 2, 512], FP16, tag="pt", name="pt")
                nc.scalar.activation(out=pt[:], in_=st[:], func=Exp, scale=SCALE)
                pending[w] = (p, qb, kb, pt)
                if kb == 0:
                    qstate[(p, qb)] = {
                        "o": opp.tile([128, 512], F32, tag="o", name="o2"),
                        "ssum": [
                            ssp.tile([128, 2, 512], FP16, tag=f"ssum{j}", name=f"ssum{j}")
                            for j in range(2)
                        ],
                    }
                for fn in filler.get(w, ()):
                    fn()
            wl = w - LAG
            if wl in pending:
                pl, ql, kl, ptl = pending.pop(wl)
                emit_pv(pl, ql, kl, ptl)
                if kl == KB - 1:
                    epilogue(pl, ql)


def build_nc():
    nc = bacc.Bacc(
        "TRN2",
        target_bir_lowering=False,
        debug=False,
        num_devices=NCORES,
        enable_partition_id=False,
    )
    xt = nc.dram_tensor("xt", [C, N], BF16, kind="ExternalInput").ap()
    wq_d = [nc.dram_tensor(f"wq{m}", [128, KC, 128], BF16, kind="ExternalInput").ap() for m in range(2)]
    wk_d = [nc.dram_tensor(f"wk{m}", [128, KC, 128], BF16, kind="ExternalInput").ap() for m in range(2)]
    wv_d = nc.dram_tensor("wvt", [C, HD], BF16, kind="ExternalInput").ap()
    bq = nc.dram_tensor("bq", [HD], F32, kind="ExternalInput").ap()
    bk = nc.dram_tensor("bk", [HD], F32, kind="ExternalInput").ap()
    bv = nc.dram_tensor("bv", [HD], F32, kind="ExternalInput").ap()
    out_o = nc.dram_tensor("out_o", [2, 128, N], BF16, kind="ExternalOutput").ap()
    out_s = nc.dram_tensor("out_s", [2, 2, N], F32, kind="ExternalOutput").ap()

    with tile.TileContext(nc) as tc:
        build_kernel(tc, xt, wq_d, wk_d, wv_d, bq, bk, bv, out_o, out_s)
    nc.compile()
    return nc


def _pack_w(w, m):
    """[C, HD] transposed weight -> pair-m packed [128, KC, 128] bf16."""
    wt = np.asarray(w, np.float32)[:, m * 128 : (m + 1) * 128]  # [C, 128]
    return np.ascontiguousarray(
        wt.reshape(KC, 128, 128).transpose(1, 0, 2)
    ).astype(ml_dtypes.bfloat16)


def shard_inputs(inputs):
    x = np.asarray(inputs["x"], np.float32)
    in_maps = []
    for core in range(NCORES):
        b, g = core // 4, core % 4
        sl = slice(g * HD, (g + 1) * HD)
        wqt = np.asarray(inputs["Wq"], np.float32)[sl, :].T  # [C, HD]
        wkt = np.asarray(inputs["Wk"], np.float32)[sl, :].T
        wvt = np.asarray(inputs["Wv"], np.float32)[sl, :].T
        in_maps.append(
            {
                "xt": np.ascontiguousarray(x[b].T).astype(ml_dtypes.bfloat16),
                "wq0": _pack_w(wqt, 0),
                "wq1": _pack_w(wqt, 1),
                "wk0": _pack_w(wkt, 0),
                "wk1": _pack_w(wkt, 1),
                "wvt": np.ascontiguousarray(wvt).astype(ml_dtypes.bfloat16),
                "bq": np.ascontiguousarray(np.asarray(inputs["bq"], np.float32)[sl]),
                "bk": np.ascontiguousarray(np.asarray(inputs["bk"], np.float32)[sl]),
                "bv": np.ascontiguousarray(np.asarray(inputs["bv"], np.float32)[sl]),
            }
        )
    return in_maps


def assemble(results, B=2):
    out = np.zeros((B, N, C), np.float32)
    for core in range(NCORES):
        b, g = core // 4, core % 4
        oo = np.asarray(results[core]["out_o"], np.float32)  # [2, 128, N]
        os_ = np.asarray(results[core]["out_s"], np.float32)  # [2, 2, N]
        o = oo.reshape(2, 2, D, N)  # [pair, head, d, n]
        on = o / os_[:, :, None, :]
        # [pair, head, d, n] -> [n, pair*2*D + head*D + d]
        out[b, :, g * HD : (g + 1) * HD] = (
            on.transpose(3, 0, 1, 2).reshape(N, HD)
        )
    return out


_NC_CACHE = None


def _get_nc():
    global _NC_CACHE
    if _NC_CACHE is None:
        _NC_CACHE = build_nc()
    return _NC_CACHE


def kernel(**inputs):
    nc = _get_nc()
    in_maps = shard_inputs(inputs)
    res = run_bass_kernel_spmd(
        nc,
        in_maps,
        core_ids=list(range(NCORES)),
        trace=bool(int(os.environ.get("KERNEL_TRACE", "0"))),
    )
    return assemble(res.results, B=int(np.asarray(inputs["x"]).shape[0]))


# revision 23
# speedup vs baseline: 1.5910x; 1.0135x over previous
"""Multi-head attention forward kernel for Trainium2 (8 NeuronCores).

Problem: B=2, N=2048, C=1024, H=16 heads, head_dim=64.
    q = x @ Wq.T + bq  (same for k, v)
    out = softmax(q k^T / sqrt(C)) v       (per head), re-merged to [B, N, C]

Sharding: core = (batch b, head-group g): b = core // 4, g = core % 4.
Each core computes 4 heads of one batch element. No collectives needed --
outputs are disjoint; host gathers and finishes with a cheap epilogue
(normalize by the row-sums and transpose).

v4 design (trace-driven evolution of the 188us baseline):
  - The EXP stream on ACT (128 ops x ~1.05us) is the hard floor; the
    kernel is one flat 128-window pipeline (window = one key chunk of one
    (pair, query-block)): QK pair -> EXP -> static filler -> lagged PV.
  - PV/ssum trail the exp stream by L=6 windows and cross query-block
    boundaries (two O^T PSUM accumulators in flight), so the PE overflow
    of the v-projection-heavy first windows spreads into pair-1's slack
    instead of starving ACT.
  - Startup: weights arrive pair-split and pre-packed in SBUF layout
    (one contiguous 256KB DMA each); x columns stream in 256/512-col
    slabs.  The first q/k projections chase the per-chunk DMAs, so the
    first EXP fires at ~13us instead of ~29us.  DMA issue order is the
    priority order (descriptors of each dma_start spread over all 16
    queues; the phase runs at aggregate HBM bandwidth).
  - kt0 blocks are computed in N=256 column sub-blocks chasing the x
    slabs; remaining projection blocks are spread as small parts over
    windows with PE slack (pair-1 carries the late qt1 blocks only).
  - Softmax denominators: DVE folds the two fp16 parity accumulators,
    then one ones-matmul per head (PSUM partitions {0,32} via
    tile_position) -- half the baseline's reduction matmuls.
  - PSUM budget (8 banks): st double-buffer 4 + two O^T accumulators 2 +
    shared proj/ones pool 2.
Outputs: out_o [2, 128, N] bf16 (pair, head-major O^T rows, queries),
         out_s [2, 2, N]   f32 (pair, head, query sums).
"""

import os
import sys

import ml_dtypes
import numpy as np

for _p in ("/opt/trn_rl_repo",):
    if _p not in sys.path:
        sys.path.insert(0, _p)

import concourse.bass as bass  # noqa: E402
import concourse.tile as tile  # noqa: E402
from concourse import bacc, mybir  # noqa: E402
from concourse.bass_utils import run_bass_kernel_spmd  # noqa: E402

N = 2048  # sequence length
C = 1024  # model dim
D = 64  # head dim
NH = 4  # heads per core
HD = NH * D  # 256 output channels per core
NCORES = 8
KB = N // 128  # 16 key chunks of 128
QB = N // 512  # 4 query blocks of 512
KC = C // 128  # 8 contraction chunks for projections
SCALE = 1.0 / 32.0  # 1 / sqrt(C)
LAG = 6  # PV/ssum windows behind the exp stream

F32 = mybir.dt.float32
BF16 = mybir.dt.bfloat16
FP16 = mybir.dt.float16


def build_kernel(tc, xt, wq_d, wk_d, wv_d, bq, bk, bv, out_o, out_s):
    nc = tc.nc
    Exp = mybir.ActivationFunctionType.Exp

    with (
        tc.tile_pool(name="res", bufs=1) as res,
        tc.tile_pool(name="ppsum", bufs=2, space="PSUM") as ppsum,
        tc.tile_pool(name="stp", bufs=2, space="PSUM") as stp,
        tc.tile_pool(name="opp", bufs=2, space="PSUM") as opp,
        tc.tile_pool(name="ptp", bufs=12) as ptp,
        tc.tile_pool(name="otp", bufs=2) as otp,
        tc.tile_pool(name="ssp", bufs=2) as ssp,
    ):
        # ---- resident SBUF tensors ----
        # weights arrive pre-packed per head-pair: [128, KC, 128]
        wq_p = [res.tile([128, KC, 128], BF16, tag=f"wq{m}", name=f"wq{m}") for m in range(2)]
        wk_p = [res.tile([128, KC, 128], BF16, tag=f"wk{m}", name=f"wk{m}") for m in range(2)]
        wv_all = res.tile([128, KC, HD], BF16, tag="wv", name="wv")
        xt_sb = [res.tile([128, N], BF16, tag=f"xt{k}", name=f"xt{k}") for k in range(KC)]
        wv_sb = [wv_all[:, k, :] for k in range(KC)]
        qt_sb = [res.tile([128, N], BF16, tag=f"qt{m}", name=f"qt{m}") for m in range(2)]
        kt_sb = [res.tile([128, N], BF16, tag=f"kt{m}", name=f"kt{m}") for m in range(2)]
        v_sb = [res.tile([128, NH, D], FP16, tag=f"v{kb}", name=f"v{kb}") for kb in range(KB)]
        bq_sb = [res.tile([128, 1], F32, tag=f"bq{m}", name=f"bq{m}") for m in range(2)]
        bk_sb = [res.tile([128, 1], F32, tag=f"bk{m}", name=f"bk{m}") for m in range(2)]
        bv_sb = res.tile([128, HD], F32, tag="bv", name="bv")
        ones_sb = res.tile([128, 1], FP16, tag="ones", name="ones")
        warm_sb = res.tile([1, 2], F32, tag="warm", name="warm")
        warmmm_sb = res.tile([128, 64], BF16, tag="warmmm", name="warmmm")

        # ---- input DMAs in strict priority order ----
        nc.sync.dma_start(out=wq_p[0][:], in_=wq_d[0])
        nc.sync.dma_start(out=wk_p[0][:], in_=wk_d[0])
        for m in range(2):
            sl = slice(m * 128, (m + 1) * 128)
            nc.sync.dma_start(out=bq_sb[m][:], in_=bq[sl])
            nc.sync.dma_start(out=bk_sb[m][:], in_=bk[sl])
        bv_bcast = bass.AP(tensor=bv.tensor, offset=bv.offset, ap=[[0, 128]] + list(bv.ap))
        nc.sync.dma_start(out=bv_sb[:], in_=bv_bcast)
        for k in range(KC):
            nc.sync.dma_start(out=xt_sb[k][:, 0:512], in_=xt[k * 128 : (k + 1) * 128, 0:512])
        nc.sync.dma_start(out=wv_all[:], in_=wv_d.rearrange("(k p) n -> p k n", p=128))
        for j in range(2, 8):  # x columns 512:2048 in 256-col slabs
            for k in range(KC):
                nc.sync.dma_start(
                    out=xt_sb[k][:, j * 256 : (j + 1) * 256],
                    in_=xt[k * 128 : (k + 1) * 128, j * 256 : (j + 1) * 256],
                )
        nc.sync.dma_start(out=wq_p[1][:], in_=wq_d[1])
        nc.sync.dma_start(out=wk_p[1][:], in_=wk_d[1])

        nc.vector.memset(ones_sb[:], 1.0)
        # warm up the ACT exp table while DMAs land
        nc.vector.memset(warm_sb[:], 0.0)
        nc.scalar.activation(out=warm_sb[:, 0:1], in_=warm_sb[:, 1:2], func=Exp)
        # warm up the PE (HAM un-throttles after ~3.4us of sustained
        # activity) on junk data so the prologue projections run at 2.4GHz
        nc.vector.memset(warmmm_sb[:], 0.5)
        wps = ppsum.tile([64, 64], F32, tag="qkps", name="wps")
        for i in range(56):
            nc.tensor.matmul(
                out=wps[:],
                lhsT=warmmm_sb[:, 0:64],
                rhs=warmmm_sb[:],
                start=(i == 0),
                stop=(i == 55),
            )

        # ---- building blocks ----
        def proj_qk_part(state, which, m, nb, k0, k1):
            """Chunks [k0, k1) of a q/k projection block [128, 512]."""
            w_p = (wq_p if which == "q" else wk_p)[m]
            nsl = slice(nb * 512, (nb + 1) * 512)
            if k0 == 0:
                state["ps"] = ppsum.tile([128, 512], F32, tag="qkps", name="qkps")
            ps = state["ps"]
            for k in range(k0, k1):
                nc.tensor.matmul(
                    out=ps[:],
                    lhsT=w_p[:, k, :],
                    rhs=xt_sb[k][:, nsl],
                    start=(k == 0),
                    stop=(k == KC - 1),
                )
            if k1 == KC:
                b_sb = (bq_sb if which == "q" else bk_sb)[m]
                t_sb = (qt_sb if which == "q" else kt_sb)[m]
                nc.vector.tensor_scalar_add(out=t_sb[:, nsl], in0=ps[:], scalar1=b_sb[:])

        def proj_qk_256(which, m, nb2):
            """One N=256 column sub-block of a q/k projection (chases the
            256-col x slabs)."""
            w_p = (wq_p if which == "q" else wk_p)[m]
            nsl = slice(nb2 * 256, (nb2 + 1) * 256)
            ps = ppsum.tile([128, 256], F32, tag="qkps", name="qkps2")
            for k in range(KC):
                nc.tensor.matmul(
                    out=ps[:],
                    lhsT=w_p[:, k, :],
                    rhs=xt_sb[k][:, nsl],
                    start=(k == 0),
                    stop=(k == KC - 1),
                )
            b_sb = (bq_sb if which == "q" else bk_sb)[m]
            t_sb = (qt_sb if which == "q" else kt_sb)[m]
            nc.vector.tensor_scalar_add(out=t_sb[:, nsl], in0=ps[:], scalar1=b_sb[:])

        def proj_v_block(kb):
            vps = ppsum.tile([128, HD], F32, tag="qkps", name="vps")
            for k in range(KC):
                nc.tensor.matmul(
                    out=vps[:],
                    lhsT=xt_sb[k][:, kb * 128 : (kb + 1) * 128],
                    rhs=wv_sb[k][:],
                    start=(k == 0),
                    stop=(k == KC - 1),
                )
            nc.vector.tensor_add(
                out=v_sb[kb][:],
                in0=vps[:].rearrange("p (h d) -> p h d", h=NH),
                in1=bv_sb[:].rearrange("p (h d) -> p h d", h=NH),
            )

        # ---- static filler schedule: window -> list of closures ----
        filler = {}

        def sched(w, fn):
            filler.setdefault(w, []).append(fn)

        def sched_parts(windows, which, m, nb):
            st = {}
            bounds = [round(i * KC / len(windows)) for i in range(len(windows) + 1)]
            for w, k0, k1 in zip(windows, bounds, bounds[1:]):
                sched(w, lambda st=st, k0=k0, k1=k1: proj_qk_part(st, which, m, nb, k0, k1))

        for kb in range(KB):  # v(kb) three windows ahead of its PV
            sched(kb + 3, lambda kb=kb: proj_v_block(kb))
        # kt0 column sub-blocks chase the x slab DMAs
        for w, nb2 in ((1, 2), (2, 3), (5, 4), (6, 5), (9, 6), (10, 7)):
            sched(w, lambda nb2=nb2: proj_qk_256("k", 0, nb2))
        sched_parts([12, 13], "q", 0, 1)
        sched_parts([20, 21, 22, 23], "q", 0, 2)
        sched_parts([33, 34, 35, 36], "q", 0, 3)
        sched_parts([38, 39, 40, 41], "k", 1, 0)
        sched_parts([43, 44, 45, 46], "k", 1, 1)
        sched_parts([48, 49, 50, 51], "k", 1, 2)
        sched_parts([53, 54, 55, 56], "k", 1, 3)
        sched_parts([58, 59, 60, 61], "q", 1, 0)
        sched_parts([72, 73, 74, 75], "q", 1, 1)
        sched_parts([88, 89, 90, 91], "q", 1, 2)
        sched_parts([104, 105, 106, 107], "q", 1, 3)

        # ---- prologue: first projections chase the per-chunk x DMAs ----
        k0ps = ppsum.tile([128, 512], F32, tag="qkps", name="k0ps")
        q0ps = ppsum.tile([128, 512], F32, tag="qkps", name="q0ps")
        for k in range(KC):
            nc.tensor.matmul(
                out=k0ps[:], lhsT=wk_p[0][:, k, :], rhs=xt_sb[k][:, 0:512],
                start=(k == 0), stop=(k == KC - 1),
            )
            nc.tensor.matmul(
                out=q0ps[:], lhsT=wq_p[0][:, k, :], rhs=xt_sb[k][:, 0:512],
                start=(k == 0), stop=(k == KC - 1),
            )
        nc.vector.tensor_scalar_add(out=kt_sb[0][:, 0:512], in0=k0ps[:], scalar1=bk_sb[0][:])
        nc.vector.tensor_scalar_add(out=qt_sb[0][:, 0:512], in0=q0ps[:], scalar1=bq_sb[0][:])

        # ---- the flat lagged window pipeline ----
        qstate = {}

        def emit_pv(p, qb, kb, pt):
            s = qstate[(p, qb)]
            o_ps = s["o"]
            for h in range(2):
                nc.tensor.matmul(
                    out=o_ps[h * D : (h + 1) * D, :],
                    lhsT=v_sb[kb][:, 2 * p + h, :],
                    rhs=pt[:, h, :],
                    start=(kb == 0),
                    stop=(kb == KB - 1),
                    tile_position=(0, h * D),
                    skip_group_check=True,
                )
            sj = s["ssum"][kb % 2]
            if kb < 2:
                nc.vector.tensor_copy(out=sj[:], in_=pt[:])
            else:
                nc.vector.tensor_add(out=sj[:], in0=sj[:], in1=pt[:])

        def epilogue(p, qb):
            s = qstate.pop((p, qb))
            qsl = slice(qb * 512, (qb + 1) * 512)
            ssum = s["ssum"]
            nc.vector.tensor_add(out=ssum[0][:], in0=ssum[0][:], in1=ssum[1][:])
            s_ps = ppsum.tile([33, 512], F32, tag="qkps", name="sps")
            for h in range(2):
                nc.tensor.matmul(
                    out=s_ps[32 * h : 32 * h + 1, :],
                    lhsT=ones_sb[:],
                    rhs=ssum[0][:, h, :],
                    start=True,
                    stop=True,
                    tile_position=(0, 32 * h),
                    skip_group_check=True,
                )
            ss = otp.tile([33, 512], F32, tag="ss", name="ss")
            for h in range(2):
                nc.vector.tensor_copy(
                    out=ss[32 * h : 32 * h + 1, :], in_=s_ps[32 * h : 32 * h + 1, :]
                )
            ss_view = bass.AP(
                tensor=ss.tensor, offset=ss.offset,
                ap=[[32 * ss.ap[0][0], 2]] + list(ss.ap[1:]),
            )
            nc.sync.dma_start(out=out_s[p, :, qsl], in_=ss_view)
            ot = otp.tile([128, 512], BF16, tag="ot", name="ot")
            nc.vector.tensor_copy(out=ot[:], in_=s["o"][:])
            nc.sync.dma_start(out=out_o[p, :, qsl], in_=ot[:])

        # Windows are processed in groups of two -- both QK pairs, then
        # both EXPs, then filler, then both lagged PVs -- so same-PE-mode
        # matmuls sit back to back and tiling-mode-switch drains happen
        # once per group instead of once per window.  The 2-slot st pool
        # still pipelines: QK(w+1) fills slot B while exp(w) reads A, and
        # QK(w+2) reuses A which exp(w) freed a full window earlier.
        windows = [(p, qb, kb) for p in range(2) for qb in range(QB) for kb in range(KB)]
        pending = {}

        def emit_qk(w):
            p, qb, kb = windows[w]
            qsl = slice(qb * 512, (qb + 1) * 512)
            ksl = slice(kb * 128, (kb + 1) * 128)
            st = stp.tile([128, 2, 512], F32, tag="st", name="st")
            for h in range(2):
                hsl = slice(h * D, (h + 1) * D)
                nc.tensor.matmul(
                    out=st[:, h, :],
                    lhsT=kt_sb[p][hsl, ksl],
                    rhs=qt_sb[p][hsl, qsl],
                    start=True,
                    stop=True,
                )
            return st

        def emit_exp(w, st):
            p, qb, kb = windows[w]
            pt = ptp.tile([128, 2, 512], FP16, tag="pt", name="pt")
            nc.scalar.activation(out=pt[:], in_=st[:], func=Exp, scale=SCALE)
            pending[w] = (p, qb, kb, pt)
            if kb == 0:
                qstate[(p, qb)] = {
                    "o": opp.tile([128, 512], F32, tag="o", name="o2"),
                    "ssum": [
                        ssp.tile([128, 2, 512], FP16, tag=f"ssum{j}", name=f"ssum{j}")
                        for j in range(2)
                    ],
                }

        def drain_pv(wl):
            if wl in pending:
                pl, ql, kl, ptl = pending.pop(wl)
                emit_pv(pl, ql, kl, ptl)
                if kl == KB - 1:
                    epilogue(pl, ql)

        assert LAG % 2 == 0
        for w in range(0, len(windows), 2):
            st_a = emit_qk(w)
            st_b = emit_qk(w + 1)
            emit_exp(w, st_a)
            emit_exp(w + 1, st_b)
            for fn in filler.get(w, ()):
                fn()
            for fn in filler.get(w + 1, ()):
                fn()
            drain_pv(w - LAG)
            drain_pv(w + 1 - LAG)
        for wl in range(len(windows) - LAG, len(windows)):
            drain_pv(wl)


def build_nc():
    nc = bacc.Bacc(
        "TRN2",
        target_bir_lowering=False,
        debug=False,
        num_devices=NCORES,
        enable_partition_id=False,
    )
    xt = nc.dram_tensor("xt", [C, N], BF16, kind="ExternalInput").ap()
    wq_d = [nc.dram_tensor(f"wq{m}", [128, KC, 128], BF16, kind="ExternalInput").ap() for m in range(2)]
    wk_d = [nc.dram_tensor(f"wk{m}", [128, KC, 128], BF16, kind="ExternalInput").ap() for m in range(2)]
    wv_d = nc.dram_tensor("wvt", [C, HD], BF16, kind="ExternalInput").ap()
    bq = nc.dram_tensor("bq", [HD], F32, kind="ExternalInput").ap()
    bk = nc.dram_tensor("bk", [HD], F32, kind="ExternalInput").ap()
    bv = nc.dram_tensor("bv", [HD], F32, kind="ExternalInput").ap()
    out_o = nc.dram_tensor("out_o", [2, 128, N], BF16, kind="ExternalOutput").ap()
    out_s = nc.dram_tensor("out_s", [2, 2, N], F32, kind="ExternalOutput").ap()

    with tile.TileContext(nc) as tc:
        build_kernel(tc, xt, wq_d, wk_d, wv_d, bq, bk, bv, out_o, out_s)
    nc.compile()
    return nc


def _pack_w(w, m):
    """[C, HD] transposed weight -> pair-m packed [128, KC, 128] bf16."""
    wt = np.asarray(w, np.float32)[:, m * 128 : (m + 1) * 128]  # [C, 128]
    return np.ascontiguousarray(
        wt.reshape(KC, 128, 128).transpose(1, 0, 2)
    ).astype(ml_dtypes.bfloat16)


def shard_inputs(inputs):
    x = np.asarray(inputs["x"], np.float32)
    in_maps = []
    for core in range(NCORES):
        b, g = core // 4, core % 4
        sl = slice(g * HD, (g + 1) * HD)
        wqt = np.asarray(inputs["Wq"], np.float32)[sl, :].T  # [C, HD]
        wkt = np.asarray(inputs["Wk"], np.float32)[sl, :].T
        wvt = np.asarray(inputs["Wv"], np.float32)[sl, :].T
        in_maps.append(
            {
                "xt": np.ascontiguousarray(x[b].T).astype(ml_dtypes.bfloat16),
                "wq0": _pack_w(wqt, 0),
                "wq1": _pack_w(wqt, 1),
                "wk0": _pack_w(wkt, 0),
                "wk1": _pack_w(wkt, 1),
                "wvt": np.ascontiguousarray(wvt).astype(ml_dtypes.bfloat16),
                "bq": np.ascontiguousarray(np.asarray(inputs["bq"], np.float32)[sl]),
                "bk": np.ascontiguousarray(np.asarray(inputs["bk"], np.float32)[sl]),
                "bv": np.ascontiguousarray(np.asarray(inputs["bv"], np.float32)[sl]),
            }
        )
    return in_maps


def assemble(results, B=2):
    out = np.zeros((B, N, C), np.float32)
    for core in range(NCORES):
        b, g = core // 4, core % 4
        oo = np.asarray(results[core]["out_o"], np.float32)  # [2, 128, N]
        os_ = np.asarray(results[core]["out_s"], np.float32)  # [2, 2, N]
        o = oo.reshape(2, 2, D, N)  # [pair, head, d, n]
        on = o / os_[:, :, None, :]
        # [pair, head, d, n] -> [n, pair*2*D + head*D + d]
        out[b, :, g * HD : (g + 1) * HD] = (
            on.transpose(3, 0, 1, 2).reshape(N, HD)
        )
    return out


_NC_CACHE = None


def _get_nc():
    global _NC_CACHE
    if _NC_CACHE is None:
        _NC_CACHE = build_nc()
    return _NC_CACHE


def kernel(**inputs):
    nc = _get_nc()
    in_maps = shard_inputs(inputs)
    res = run_bass_kernel_spmd(
        nc,
        in_maps,
        core_ids=list(range(NCORES)),
        trace=bool(int(os.environ.get("KERNEL_TRACE", "0"))),
    )
    return assemble(res.results, B=int(np.asarray(inputs["x"]).shape[0]))
